# revision 40
# baseline (speedup 1.0000x reference)
"""Distributed multi-head attention kernel for 8 TRN2 NeuronCores.

Module: B=2, N=2048, D_MODEL=1024, H=16, D_HEAD=64 attention with
arbitrary rotary embedding, key-side boolean masking, softmax, and
output projection.

Sharding: head-parallel attention (2 heads per core, both batches).
v7: NO collective.  Each core applies its own 128-channel slice of
Wout to its normalized attention output per pass and ships a partial
[4096, 1024] product; the host sums the 8 partials and adds bout.
This removes the v6 tail (a2a_in DMA + 19us collective trigger
latency + 46us AllToAll + 37us phase-3 gather/projection).

 - Projections (phase 1) are EMITTED INTERLEAVED with the attention
   passes; the Tile scheduler fills the PE's idle time during the
   ACT-bound softmax stream with the next row-block's projection
   matmuls.
 - Attention software-pipelined per 512-q-row pass: both heads'
   score blocks share one [128,1024] PSUM tile (the two K=64 score
   matmuls auto-pack into row groups 0-1/2-3 and run concurrently),
   one exp per key tile covers both heads, per-kt emission order is
   score -> exp -> fillers -> attnV so the exp stream never waits on
   filler PE work.
 - Rotary on device: rot2(q) = ProtT.T @ q (constant +-1 permutation
   matmul) instead of host-rotated duplicate weight projections.
 - Softmax denominators via a ones-column in V (lhsT = [v | 1], M=65);
   key mask folded into the exp as a per-partition bias.
 - Per-pass tail (normalization + 8 output-projection matmuls + out
   DMA) is woven into the NEXT pass as its first 6 filler slots, so
   the o-accumulator PSUM banks release before attnV(kt=0) of the
   next pass and the PE absorbs the y matmuls under the exp stream.
"""
import os
import warnings

warnings.filterwarnings("ignore")
import numpy as np
import ml_dtypes

from concourse import bacc, tile, mybir, bass_utils

B, N, DM, H, DH = 2, 2048, 1024, 16, 64
R = B * N
NCORES = 8
HPC = 2
CPC = HPC * DH       # 128 chans per core
KT = 8               # contraction tiles over d_model
RB = 8               # row blocks of 512 over R
NKEYT = 16           # key tiles of 128 over N
QC = 512             # q rows per attention pass
NPASS = N // QC      # 4 passes per batch
NPT = B * NPASS      # 8 passes total

F32 = mybir.dt.float32
BF16 = mybir.dt.bfloat16

VAUGW = 2 * (DH + 1)      # 130 cols per key tile: [vA | 1 | vB | 1]

LAST_EXEC_TIME_NS = None
LAST_TRACE_DIR = None


def _install_trace_shim():
    import sys
    import types
    import ctypes
    import contextlib

    if "antenv.axon_hooks" in sys.modules:
        return
    so_path = "/opt/axon/libaxon_pjrt.so"
    hook = None
    if os.path.exists(so_path):
        lib = ctypes.CDLL(so_path)
        if hasattr(lib, "axon_start_nrt_profile"):
            lib.axon_start_nrt_profile.argtypes = [
                ctypes.POINTER(ctypes.c_int64), ctypes.c_size_t]
            lib.axon_start_nrt_profile.restype = ctypes.c_int64
            lib.axon_stop_nrt_profile.argtypes = [ctypes.c_char_p]
            lib.axon_stop_nrt_profile.restype = ctypes.c_int64

            @contextlib.contextmanager
            def _hook(output_dir, device_ids):
                import jax
                jax.devices()
                if device_ids:
                    ids = (ctypes.c_int64 * len(device_ids))(*device_ids)
                    rc = lib.axon_start_nrt_profile(ids, len(device_ids))
                else:
                    rc = lib.axon_start_nrt_profile(None, 0)
                if rc != 0:
                    raise RuntimeError(f"axon_start_nrt_profile rc={rc}")
                try:
                    yield
                finally:
                    n = lib.axon_stop_nrt_profile(str(output_dir).encode())
                    print(f"[trace] {n} profile file(s) -> {output_dir}")

            hook = _hook

    mod = types.ModuleType("antenv.axon_hooks")
    mod.get_axon_ntff_profile_hook = lambda: hook
    mod.set_axon_ntff_profile_hook = lambda h: None
    sys.modules["antenv.axon_hooks"] = mod
    bass_utils.upload_artifacts = lambda tmpdir: tmpdir


def build(dbg=False):
    nc = bacc.Bacc("TRN2", target_bir_lowering=False, debug=False,
                   num_devices=NCORES)

    # xt / projection weights arrive HOST-REARRANGED so every DMA reads
    # 2-8 KB contiguous per partition line (strided 1 KB lines measured
    # ~35 GB/s/queue vs ~98 GB/s for large-line transfers).
    xt_d = nc.dram_tensor("xt", [128, RB * KT * 512], BF16,
                          kind="ExternalInput")
    wq_d = nc.dram_tensor("wq", [128, KT * CPC], BF16, kind="ExternalInput")
    wk_d = nc.dram_tensor("wk", [128, KT * CPC], BF16, kind="ExternalInput")
    wv_d = nc.dram_tensor("wv", [128, KT * CPC], BF16, kind="ExternalInput")
    prot_d = nc.dram_tensor("prot", [128, 128], BF16, kind="ExternalInput")
    wout_d = nc.dram_tensor("wout", [CPC, DM], BF16, kind="ExternalInput")
    cost_d = nc.dram_tensor("cost", [CPC, N], BF16, kind="ExternalInput")
    sint_d = nc.dram_tensor("sint", [CPC, N], BF16, kind="ExternalInput")
    maskb_d = nc.dram_tensor("maskb", [128, R // 128], F32, kind="ExternalInput")
    vones_d = nc.dram_tensor("vones", [128, (R // 128) * 2], BF16,
                             kind="ExternalInput")

    out_d = nc.dram_tensor("out", [R, DM], BF16, kind="ExternalOutput")

    scale = float(DH ** -0.5)

    with tile.TileContext(nc) as tc:
        with tc.tile_pool(name="persist", bufs=1) as pp:
            wq_sb = pp.tile([128, KT, CPC], BF16, tag="wq")
            wk_sb = pp.tile([128, KT, CPC], BF16, tag="wk")
            wv_sb = pp.tile([128, KT, CPC], BF16, tag="wv")
            prot_sb = pp.tile([128, 128], BF16, tag="prot")
            cost_sb = pp.tile([CPC, N], BF16, tag="cost")
            sint_sb = pp.tile([CPC, N], BF16, tag="sint")
            maskb_sb = pp.tile([128, R // 128], F32, tag="maskb")
            qt_sb = pp.tile([CPC, R], BF16, tag="qt")
            kt_sb = pp.tile([CPC, R], BF16, tag="kt")
            vaug_sb = pp.tile([128, (R // 128) * VAUGW], BF16, tag="vaug")
            wo_sb = pp.tile([128, DM], BF16, tag="wo")
            ones_sb = pp.tile([128, 128], BF16, tag="ones")
            nc.vector.memset(ones_sb[:], 1.0)

            junk_sb = pp.tile([128, 512], BF16, tag="junk")
            nc.vector.memset(junk_sb[:], 0.001)

            xt_view = xt_d.ap().rearrange("p (rb k n) -> p rb k n",
                                          rb=RB, k=KT)

            # ALL xt row blocks are SBUF-resident (8 MB); every input DMA
            # is issued up front, striped over the three trigger queues,
            # ordered so the earliest-needed bytes land first.
            xt_all = pp.tile([128, RB, KT, 512], BF16, tag="xtall")
            woB2 = pp.tile([DH, DM], BF16, tag="woB2")
            # Queue order = landing order.  Critical path to the first
            # exp: wq/wk -> rb0 (split over two queues) -> cost/sint
            # first block -> maskb.
            nc.sync.dma_start(wq_sb[:],
                              wq_d.ap().rearrange("p (k n) -> p k n", k=KT))
            nc.scalar.dma_start(wk_sb[:],
                                wk_d.ap().rearrange("p (k n) -> p k n", k=KT))
            nc.gpsimd.dma_start(wv_sb[:],
                                wv_d.ap().rearrange("p (k n) -> p k n", k=KT))
            nc.sync.dma_start(xt_all[:, 0, 0:4], xt_view[:, 0, 0:4])
            nc.gpsimd.dma_start(xt_all[:, 0, 4:8], xt_view[:, 0, 4:8])
            nc.scalar.dma_start(cost_sb[:, 0:512], cost_d[:, 0:512])
            nc.scalar.dma_start(sint_sb[:, 0:512], sint_d[:, 0:512])
            nc.scalar.dma_start(prot_sb[:], prot_d[:, :])
            # pre-load the ACT Exp table during the initial DMA wait
            warm_sb = pp.tile([1, 2], F32, tag="warm")
            nc.vector.memset(warm_sb[:], 0.0)
            nc.scalar.activation(warm_sb[0:1, 1:2], warm_sb[0:1, 0:1],
                                 mybir.ActivationFunctionType.Exp)
            nc.sync.dma_start(maskb_sb[:], maskb_d[:, :])
            ones_view = vaug_sb[:].rearrange("p (t u w) -> p (t u) w",
                                             u=2, w=DH + 1)[:, :, DH]
            nc.gpsimd.dma_start(ones_view, vones_d[:, :])
            nc.sync.dma_start(xt_all[:, 1], xt_view[:, 1])
            nc.gpsimd.dma_start(xt_all[:, 2], xt_view[:, 2])
            nc.scalar.dma_start(xt_all[:, 3], xt_view[:, 3])
            nc.scalar.dma_start(cost_sb[:, 512:], cost_d[:, 512:])
            nc.scalar.dma_start(sint_sb[:, 512:], sint_d[:, 512:])
            nc.sync.dma_start(xt_all[:, 4], xt_view[:, 4])
            nc.gpsimd.dma_start(xt_all[:, 5], xt_view[:, 5])
            nc.sync.dma_start(wo_sb[:], wout_d[:, :])
            nc.gpsimd.dma_start(woB2[:], wout_d[DH:2 * DH, :])
            nc.scalar.dma_start(xt_all[:, 6], xt_view[:, 6])
            nc.sync.dma_start(xt_all[:, 7], xt_view[:, 7])

            with tc.tile_pool(name="p1", bufs=3) as p1, \
                 tc.tile_pool(name="psc", bufs=2, space="PSUM") as psc, \
                 tc.tile_pool(name="p2", bufs=3) as p2, \
                 tc.tile_pool(name="ppt", bufs=6) as ppt, \
                 tc.tile_pool(name="ps_sc", bufs=2, space="PSUM") as ps_sc, \
                 tc.tile_pool(name="ps_o", bufs=1, space="PSUM") as ps_o:

                # Warm-up stream: dependency-free junk matmuls bridge the
                # initial input-DMA wait so the PE HAM clock is at 8/8
                # when the first projection matmul issues.  All into ONE
                # tile: same-engine WAW needs no semaphore, so they run
                # back-to-back.
                JW = int(os.environ.get("BASS_JW", "30"))
                jp = psc.tile([128, 512], F32, tag="c", name="junk")
                for _ in range(JW):
                    nc.tensor.matmul(jp[:], ones_sb[:], junk_sb[:],
                                     start=True, stop=True)

                def rb_fillers(rb):
                    """Projection + rotary + v_aug for one 512-row block,
                    split into small chunks so they can be woven between
                    a pass's key-tile groups."""
                    c0 = rb * 512
                    st = {}

                    def f_start():
                        st['xt'] = xt_all[:, rb]
                        st['q'] = psc.tile([128, 512], F32, tag="c", name="q")

                    def f_q(k0):
                        def f():
                            for kt in range(k0, k0 + 4):
                                nc.tensor.matmul(
                                    st['q'][:], wq_sb[:, kt, :],
                                    st['xt'][:, kt, :],
                                    start=(kt == 0), stop=(kt == KT - 1))
                            if k0 + 4 == KT:
                                st['qraw'] = p1.tile([128, 512], BF16,
                                                     tag="qraw", name="qraw")
                                nc.vector.tensor_copy(st['qraw'][:],
                                                      st['q'][:])
                        return f

                    def f_k(k0):
                        def f():
                            if k0 == 0:
                                st['k'] = psc.tile([128, 512], F32, tag="c",
                                                   name="k")
                            for kt in range(k0, k0 + 4):
                                nc.tensor.matmul(
                                    st['k'][:], wk_sb[:, kt, :],
                                    st['xt'][:, kt, :],
                                    start=(kt == 0), stop=(kt == KT - 1))
                            if k0 + 4 == KT:
                                st['kraw'] = p1.tile([128, 512], BF16,
                                                     tag="kraw", name="kraw")
                                nc.vector.tensor_copy(st['kraw'][:],
                                                      st['k'][:])
                        return f

                    def f_v(k0):
                        def f():
                            if k0 == 0:
                                st['v'] = psc.tile([128, 512], F32, tag="c",
                                                   name="v")
                            for kt in range(k0, k0 + 4):
                                for vt in range(4):
                                    nc.tensor.matmul(
                                        st['v'][:, vt * 128:(vt + 1) * 128],
                                        st['xt'][:, kt, vt * 128:(vt + 1) * 128],
                                        wv_sb[:, kt, :],
                                        start=(kt == 0 and vt == 0),
                                        stop=(kt == KT - 1))
                            if k0 + 4 == KT:
                                kt0 = rb * 4
                                va = vaug_sb[:].rearrange("p (t w) -> p t w",
                                                          w=VAUGW)
                                vp = st['v'][:].rearrange("p (t c) -> p t c",
                                                          c=128)
                                nc.vector.tensor_copy(
                                    va[:, kt0:kt0 + 4, 0:DH], vp[:, :, 0:DH])
                                nc.vector.tensor_copy(
                                    va[:, kt0:kt0 + 4, DH + 1:DH + 1 + DH],
                                    vp[:, :, DH:2 * DH])
                        return f

                    def f_rot(dst, rawkey):
                        def f():
                            raw = st[rawkey]
                            rot_ps = psc.tile([128, 512], F32, tag="c",
                                              name="rot")
                            nc.tensor.matmul(rot_ps[:], prot_sb[:], raw[:],
                                             start=True, stop=True)
                            cc = c0 % N
                            dv = dst[:, c0:c0 + 512]
                            tmp = p1.tile([128, 512], BF16, tag="rottmp")
                            nc.vector.tensor_mul(dv, raw[:],
                                                 cost_sb[:, cc:cc + 512])
                            nc.vector.tensor_mul(tmp[:], rot_ps[:],
                                                 sint_sb[:, cc:cc + 512])
                            nc.vector.tensor_add(dv, dv, tmp[:])
                        return f

                    def f_first():
                        f_start()
                        f_q(0)()
                    return [f_first, f_q(4), f_k(0), f_k(4),
                            f_v(0), f_v(4),
                            f_rot(qt_sb, 'qraw'), f_rot(kt_sb, 'kraw')]

                def emit_rb(rb):
                    for f in rb_fillers(rb):
                        f()

                Y_ENG = {0: nc.sync, 1: nc.gpsimd, 2: nc.sync, 3: nc.gpsimd}

                def tail_fillers(j, o_ps):
                    """Normalization + local output projection for a
                    finished pass.  First chunk releases the o PSUM
                    banks (it holds every read of o_ps), so it MUST be
                    emitted before the next pass's first attnV.  All
                    engine ops keep in/out base partitions aligned; the
                    head-B 64->128 partition stack goes through one
                    small SBUF->SBUF DMA."""
                    rows0 = j * QC
                    st = {}

                    def f_readout():
                        st['rcp'] = []
                        st['onum'] = []
                        for h in range(HPC):
                            rcp = p2.tile([DH + 1, QC], F32, tag=f"rcp{h}",
                                          name=f"rcp{h}")
                            nc.vector.reciprocal_approx_fast(rcp[:], o_ps[h][:])
                            st['rcp'].append(rcp)
                            onum = p2.tile([DH, QC], BF16, tag=f"on{h}",
                                           name=f"on{h}")
                            nc.vector.tensor_copy(onum[:], o_ps[h][0:DH, :])
                            st['onum'].append(onum)

                    def f_div():
                        st['onb'] = p2.tile([128, QC], BF16, tag="onb2",
                                            name="onb2")
                        for h in range(HPC):
                            rcpb = p2.tile([DH + 1, QC], BF16, tag=f"rb{h}",
                                           name=f"rb{h}")
                            nc.vector.tensor_copy(rcpb[DH:DH + 1, :],
                                                  st['rcp'][h][DH:DH + 1, :])
                            div_ps = psc.tile([128, QC], F32, tag="c",
                                              name="div")
                            nc.tensor.matmul(div_ps[:], ones_sb[DH:DH + 1, :],
                                             rcpb[DH:DH + 1, :],
                                             start=True, stop=True,
                                             tile_position=(64, 0))
                            div_sb = p2.tile([DH, QC], BF16, tag=f"dv{h}",
                                             name=f"dv{h}")
                            nc.vector.tensor_copy(div_sb[:], div_ps[0:DH, :])
                            if h == 0:
                                nc.vector.tensor_mul(
                                    st['onb'][0:DH, :], st['onum'][0][:],
                                    div_sb[:])
                            else:
                                onbB = p2.tile([DH, QC], BF16, tag="onbB",
                                               name="onbB")
                                nc.vector.tensor_mul(onbB[:], st['onum'][1][:],
                                                     div_sb[:])
                                nc.gpsimd.dma_start(
                                    st['onb'][DH:2 * DH, :], onbB[:])

                    def f_y(i):
                        def f():
                            ysb = p2.tile([128, DM], BF16, tag="ysb",
                                          name="ysb")
                            for ob in range(2):
                                yp = psc.tile([128, 512], F32, tag="c",
                                              name="y")
                                nc.tensor.matmul(
                                    yp[:],
                                    st['onb'][:, i * 128:(i + 1) * 128],
                                    wo_sb[:, ob * 512:(ob + 1) * 512],
                                    start=True, stop=True)
                                nc.vector.tensor_copy(
                                    ysb[:, ob * 512:(ob + 1) * 512], yp[:])
                            r0 = rows0 + i * 128
                            Y_ENG[i].dma_start(out_d[r0:r0 + 128, :], ysb[:])
                        return f

                    return [f_readout, f_div, f_y(0), f_y(1), f_y(2), f_y(3)]

                def tail_last(j, o_ps):
                    """Chunked tail for the final pass: nothing overlaps
                    it, so shorten the serial chain.  Per-head K=64
                    output matmuls (accumulating pairs against
                    wo_sb[0:64] / woB2) avoid the SBUF->SBUF partition
                    stack DMA entirely."""
                    rows0 = j * QC
                    st = {}

                    def f_readout():
                        st['rcp'] = []
                        st['onum'] = []
                        for h in range(HPC):
                            rcp = p2.tile([DH + 1, QC], F32, tag=f"rcp{h}",
                                          name=f"rcp{h}")
                            nc.vector.reciprocal_approx_fast(rcp[:], o_ps[h][:])
                            st['rcp'].append(rcp)
                            onum = p2.tile([DH, QC], BF16, tag=f"on{h}",
                                           name=f"on{h}")
                            nc.vector.tensor_copy(onum[:], o_ps[h][0:DH, :])
                            st['onum'].append(onum)

                    def f_norm():
                        st['onb'] = []
                        for h in range(HPC):
                            rcpb = p2.tile([DH + 1, QC], BF16, tag=f"rb{h}",
                                           name=f"rb{h}")
                            nc.vector.tensor_copy(rcpb[DH:DH + 1, :],
                                                  st['rcp'][h][DH:DH + 1, :])
                            div_ps = psc.tile([128, QC], F32, tag="c",
                                              name="div")
                            nc.tensor.matmul(div_ps[:], ones_sb[DH:DH + 1, :],
                                             rcpb[DH:DH + 1, :],
                                             start=True, stop=True,
                                             tile_position=(64, 0))
                            div_sb = p2.tile([DH, QC], BF16, tag=f"dv{h}",
                                             name=f"dv{h}")
                            nc.vector.tensor_copy(div_sb[:], div_ps[0:DH, :])
                            onb = p2.tile([DH, QC], BF16, tag=f"onbl{h}",
                                          name=f"onbl{h}")
                            nc.vector.tensor_mul(onb[:], st['onum'][h][:],
                                                 div_sb[:])
                            st['onb'].append(onb)

                    def f_ych(i):
                        def f():
                            ysb = p2.tile([128, DM], BF16, tag="ysb",
                                          name="ysb")
                            r0 = rows0 + i * 128
                            for ob in range(2):
                                yp = psc.tile([128, 512], F32, tag="c",
                                              name="y")
                                nc.tensor.matmul(
                                    yp[:],
                                    st['onb'][0][:, i * 128:(i + 1) * 128],
                                    wo_sb[0:DH, ob * 512:(ob + 1) * 512],
                                    start=True, stop=False)
                                nc.tensor.matmul(
                                    yp[:],
                                    st['onb'][1][:, i * 128:(i + 1) * 128],
                                    woB2[:, ob * 512:(ob + 1) * 512],
                                    start=False, stop=True)
                                nc.vector.tensor_copy(
                                    ysb[:, ob * 512:(ob + 1) * 512], yp[:])
                                # half-row DMA as soon as each cast lands
                                Y_ENG[(2 * i + ob) % 2].dma_start(
                                    out_d[r0:r0 + 128,
                                          ob * 512:(ob + 1) * 512],
                                    ysb[:, ob * 512:(ob + 1) * 512])
                        return f

                    def f_warm():
                        # keep the PE HAM clock at 8/8 through the
                        # DVE-only normalization window so the final
                        # output-projection matmuls run at full rate
                        tj = psc.tile([128, 512], F32, tag="c", name="tj")
                        for _ in range(18):
                            nc.tensor.matmul(tj[:], ones_sb[:], junk_sb[:],
                                             start=True, stop=True)

                    return [f_warm, f_readout, f_norm, f_ych(0), f_ych(1),
                            f_ych(2), f_ych(3)]

                # ---- unified software-pipelined attention stream ----
                # One global loop over key-tile index G (pass p = G//16).
                # Per-G emission order: score(G) -> exp(G) -> fillers(G)
                # -> attnV(G-L).  attnV lags L=4 iterations behind the
                # score/exp front so the next pass's first score matmuls
                # sit ahead of the previous pass's attnV backlog in the
                # PE queue and the exp stream stays dense across pass
                # boundaries.
                L = 4
                GT = B * NPASS * NKEYT  # 128
                sched = {}

                def add(G, fs):
                    sched.setdefault(G, []).extend(fs)

                def pack(G0, chunks, sizes):
                    i = 0
                    g = G0
                    for s in sizes:
                        grp = chunks[i:i + s]
                        i += s
                        if not grp:
                            break

                        def runner(grp=grp):
                            for f in grp:
                                f()
                        add(g, [runner])
                        g += 1
                    assert i >= len(chunks)

                # rb1-4 woven into pass 0 (rb_i complete before G=4*i);
                # rb5-7 into passes 1-3, avoiding each pass's last slot
                # and the tail slots (4..9).
                def f_bridge():
                    # dependency-free warm-keeper ahead of the DMA-gated
                    # rb1 fillers: bridges the known pass-0 input stall
                    # so the post-stall rot/score chain runs at 8/8
                    bj = psc.tile([128, 512], F32, tag="c", name="bj")
                    for _ in range(12):
                        nc.tensor.matmul(bj[:], ones_sb[:], junk_sb[:],
                                         start=True, stop=True)
                add(1, [f_bridge])
                pack(1, rb_fillers(1), [3, 3, 2])
                pack(4, rb_fillers(2), [2, 2, 2, 2])
                pack(8, rb_fillers(3), [2, 2, 2, 2])
                pack(12, rb_fillers(4), [2, 2, 2, 2])
                for i, rb in enumerate((5, 6, 7)):
                    base = 16 * (i + 1)
                    ch = rb_fillers(rb)
                    pack(base + 1, ch[0:3], [1, 1, 1])
                    pack(base + 10, ch[3:8], [1, 1, 1, 1, 1])

                o_ps_map = {}
                pt_hist = {}

                def emit_score(p, k):
                    b, qc = divmod(p, NPASS)
                    qb = b * N + qc * QC
                    g = b * NKEYT + k
                    krow = b * N + k * 128
                    sc = ps_sc.tile([128, 2 * QC], F32, tag="sc", name="sc")
                    for h in range(HPC):
                        ho = h * DH
                        nc.tensor.matmul(
                            sc[:, h * QC:(h + 1) * QC],
                            kt_sb[ho:ho + DH, krow:krow + 128],
                            qt_sb[ho:ho + DH, qb:qb + QC],
                            start=True, stop=True)
                    pt = ppt.tile([128, 2 * QC], BF16, tag="p", name="pt")
                    nc.scalar.activation(
                        pt[:], sc[:], mybir.ActivationFunctionType.Exp,
                        bias=maskb_sb[:, g:g + 1], scale=scale)
                    pt_hist[p * NKEYT + k] = pt

                def emit_attnv(p, k):
                    b, qc = divmod(p, NPASS)
                    gp = b * NKEYT + k
                    pt = pt_hist.pop(p * NKEYT + k)
                    for h in range(HPC):
                        va_l = vaug_sb[:, gp * VAUGW + h * (DH + 1):
                                       gp * VAUGW + (h + 1) * (DH + 1)]
                        nc.tensor.matmul(
                            o_ps_map[p][h][:], va_l,
                            pt[:, h * QC:(h + 1) * QC],
                            start=(k == 0), stop=(k == NKEYT - 1))

                emit_rb(0)
                for G in range(GT + L + 8):
                    p, k = divmod(G, NKEYT)
                    if G < GT:
                        if k == 0:
                            o_ps_map[p] = [
                                ps_o.tile([DH + 1, QC], F32, tag=f"o{h}",
                                          name=f"o{h}") for h in range(HPC)]
                            # tail of pass p-1 into slots L..L+5 of pass p
                            if p > 0:
                                for i, f in enumerate(
                                        tail_fillers(p - 1, o_ps_map[p - 1])):
                                    add(G + L + i, [f])
                        emit_score(p, k)
                    if G == GT:
                        for i, f in enumerate(
                                tail_last(NPT - 1, o_ps_map[NPT - 1])):
                            add(G + L + i, [f])
                    for f in sched.pop(G, ()):
                        f()
                    if 0 <= G - L < GT:
                        p2_, k2 = divmod(G - L, NKEYT)
                        emit_attnv(p2_, k2)

    nc.compile()
    return nc


_NC_CACHE = None


def kernel(x, mask, pos_emb, Wq, Wkv, Wout, bout):
    global LAST_EXEC_TIME_NS, LAST_TRACE_DIR, _NC_CACHE

    x = np.asarray(x, dtype=np.float32)
    mask = np.asarray(mask)
    pos_emb = np.asarray(pos_emb, dtype=np.float32)
    Wq = np.asarray(Wq, dtype=np.float32)
    Wkv = np.asarray(Wkv, dtype=np.float32)
    Wout = np.asarray(Wout, dtype=np.float32)
    bout = np.asarray(bout, dtype=np.float32)

    bf = ml_dtypes.bfloat16
    # xt2[p, ((rb*KT)+kt)*512+n] = x[rb*512+n, kt*128+p]: each partition
    # line is 8 KB contiguous per row-block -> fast DMA.
    xt2 = np.ascontiguousarray(
        x.reshape(RB, 512, KT, 128).transpose(3, 0, 2, 1)
        .reshape(128, RB * KT * 512)).astype(bf)

    def wprep(w):
        # w2[p, kt*CPC+m] = w[kt*128+p, m] (2 KB contiguous lines)
        return np.ascontiguousarray(
            w.reshape(KT, 128, CPC).transpose(1, 0, 2)
            .reshape(128, KT * CPC)).astype(bf)

    wk_full = Wkv[:, :H * DH]
    wv_full = Wkv[:, H * DH:]
    cost = np.ascontiguousarray(np.tile(np.cos(pos_emb).T, (HPC, 1))).astype(bf)
    sint = np.ascontiguousarray(np.tile(np.sin(pos_emb).T, (HPC, 1))).astype(bf)
    maskb = np.ascontiguousarray(
        np.where(mask.reshape(R), 0.0, -1e5).astype(np.float32)
        .reshape(R // 128, 128).T)
    # rot2 as a matmul: rot2(q) = P @ q (q in [chan, row] layout);
    # lhsT for the tensor engine is P.T
    prot = np.zeros((128, 128), dtype=bf)
    for i in range(64):
        prot[2 * i + 1, 2 * i] = -1.0
        prot[2 * i, 2 * i + 1] = 1.0

    in_maps = []
    for c in range(NCORES):
        cols = slice(c * CPC, (c + 1) * CPC)
        in_maps.append({
            "xt": xt2,
            "wq": wprep(Wq[:, cols]),
            "wk": wprep(wk_full[:, cols]),
            "wv": wprep(wv_full[:, cols]),
            "prot": prot,
            "wout": np.ascontiguousarray(Wout[cols, :]).astype(bf),
            "cost": cost,
            "sint": sint,
            "maskb": maskb,
            "vones": np.ones((128, (R // 128) * 2), dtype=bf),
        })

    dbg = bool(int(os.environ.get("BASS_KERNEL_DEBUG", "0")))
    if _NC_CACHE is None:
        _NC_CACHE = build(dbg=dbg)
    nc = _NC_CACHE

    trace = bool(int(os.environ.get("BASS_KERNEL_TRACE", "0")))
    kwargs = {}
    if trace:
        _install_trace_shim()
        tdir = os.environ.get("BASS_TRACE_DIR", "/tmp/bass_trace_out")
        import shutil
        shutil.rmtree(tdir, ignore_errors=True)
        os.makedirs(tdir, exist_ok=True)
        kwargs["tmpdir"] = tdir
    res = bass_utils.run_bass_kernel_spmd(
        nc, in_maps, core_ids=list(range(NCORES)), trace=trace, **kwargs)
    LAST_EXEC_TIME_NS = res.exec_time_ns
    if res.instructions_and_trace is not None:
        LAST_TRACE_DIR = res.instructions_and_trace[1]
        globals()["LAST_INSTS"] = res.instructions_and_trace[0]

    globals()["LAST_RESULTS"] = res.results
    y = np.zeros((R, DM), dtype=np.float32)
    for c in range(NCORES):
        y += res.results[c]["out"].astype(np.float32)
    y += bout[None, :]
    return y.reshape(B, N, DM)


# revision 41
# speedup vs baseline: 1.0300x; 1.0300x over previous
"""Distributed multi-head attention kernel for 8 TRN2 NeuronCores.

Module: B=2, N=2048, D_MODEL=1024, H=16, D_HEAD=64 attention with
arbitrary rotary embedding, key-side boolean masking, softmax, and
output projection.

Sharding: head-parallel attention (2 heads per core, both batches).
v7: NO collective.  Each core applies its own 128-channel slice of
Wout to its normalized attention output per pass and ships a partial
[4096, 1024] product; the host sums the 8 partials and adds bout.
This removes the v6 tail (a2a_in DMA + 19us collective trigger
latency + 46us AllToAll + 37us phase-3 gather/projection).

 - Projections (phase 1) are EMITTED INTERLEAVED with the attention
   passes; the Tile scheduler fills the PE's idle time during the
   ACT-bound softmax stream with the next row-block's projection
   matmuls.
 - Attention software-pipelined per 512-q-row pass: both heads'
   score blocks share one [128,1024] PSUM tile (the two K=64 score
   matmuls auto-pack into row groups 0-1/2-3 and run concurrently),
   one exp per key tile covers both heads, per-kt emission order is
   score -> exp -> fillers -> attnV so the exp stream never waits on
   filler PE work.
 - Rotary on device: rot2(q) = ProtT.T @ q (constant +-1 permutation
   matmul) instead of host-rotated duplicate weight projections.
 - Softmax denominators via a ones-column in V (lhsT = [v | 1], M=65);
   key mask folded into the exp as a per-partition bias.
 - Per-pass tail (normalization + 8 output-projection matmuls + out
   DMA) is woven into the NEXT pass as its first 6 filler slots, so
   the o-accumulator PSUM banks release before attnV(kt=0) of the
   next pass and the PE absorbs the y matmuls under the exp stream.
"""
import os
import warnings

warnings.filterwarnings("ignore")
import numpy as np
import ml_dtypes

from concourse import bacc, tile, mybir, bass_utils

B, N, DM, H, DH = 2, 2048, 1024, 16, 64
R = B * N
NCORES = 8
HPC = 2
CPC = HPC * DH       # 128 chans per core
KT = 8               # contraction tiles over d_model
RB = 8               # row blocks of 512 over R
NKEYT = 16           # key tiles of 128 over N
QC = 512             # q rows per attention pass
NPASS = N // QC      # 4 passes per batch
NPT = B * NPASS      # 8 passes total

F32 = mybir.dt.float32
BF16 = mybir.dt.bfloat16

VAUGW = 2 * (DH + 1)      # 130 cols per key tile: [vA | 1 | vB | 1]

LAST_EXEC_TIME_NS = None
LAST_TRACE_DIR = None


def _install_trace_shim():
    import sys
    import types
    import ctypes
    import contextlib

    if "antenv.axon_hooks" in sys.modules:
        return
    so_path = "/opt/axon/libaxon_pjrt.so"
    hook = None
    if os.path.exists(so_path):
        lib = ctypes.CDLL(so_path)
        if hasattr(lib, "axon_start_nrt_profile"):
            lib.axon_start_nrt_profile.argtypes = [
                ctypes.POINTER(ctypes.c_int64), ctypes.c_size_t]
            lib.axon_start_nrt_profile.restype = ctypes.c_int64
            lib.axon_stop_nrt_profile.argtypes = [ctypes.c_char_p]
            lib.axon_stop_nrt_profile.restype = ctypes.c_int64

            @contextlib.contextmanager
            def _hook(output_dir, device_ids):
                import jax
                jax.devices()
                if device_ids:
                    ids = (ctypes.c_int64 * len(device_ids))(*device_ids)
                    rc = lib.axon_start_nrt_profile(ids, len(device_ids))
                else:
                    rc = lib.axon_start_nrt_profile(None, 0)
                if rc != 0:
                    raise RuntimeError(f"axon_start_nrt_profile rc={rc}")
                try:
                    yield
                finally:
                    n = lib.axon_stop_nrt_profile(str(output_dir).encode())
                    print(f"[trace] {n} profile file(s) -> {output_dir}")

            hook = _hook

    mod = types.ModuleType("antenv.axon_hooks")
    mod.get_axon_ntff_profile_hook = lambda: hook
    mod.set_axon_ntff_profile_hook = lambda h: None
    sys.modules["antenv.axon_hooks"] = mod
    bass_utils.upload_artifacts = lambda tmpdir: tmpdir


def build(dbg=False):
    nc = bacc.Bacc("TRN2", target_bir_lowering=False, debug=False,
                   num_devices=NCORES)

    # xt / projection weights arrive HOST-REARRANGED so every DMA reads
    # 2-8 KB contiguous per partition line (strided 1 KB lines measured
    # ~35 GB/s/queue vs ~98 GB/s for large-line transfers).
    xt_d = nc.dram_tensor("xt", [128, RB * KT * 512], BF16,
                          kind="ExternalInput")
    wq_d = nc.dram_tensor("wq", [128, KT * CPC], BF16, kind="ExternalInput")
    wk_d = nc.dram_tensor("wk", [128, KT * CPC], BF16, kind="ExternalInput")
    wv_d = nc.dram_tensor("wv", [128, KT * CPC], BF16, kind="ExternalInput")
    prot_d = nc.dram_tensor("prot", [128, 128], BF16, kind="ExternalInput")
    wout_d = nc.dram_tensor("wout", [CPC, DM], BF16, kind="ExternalInput")
    cost_d = nc.dram_tensor("cost", [CPC, N], BF16, kind="ExternalInput")
    sint_d = nc.dram_tensor("sint", [CPC, N], BF16, kind="ExternalInput")
    maskb_d = nc.dram_tensor("maskb", [128, R // 128], F32, kind="ExternalInput")
    vones_d = nc.dram_tensor("vones", [128, (R // 128) * 2], BF16,
                             kind="ExternalInput")

    out_d = nc.dram_tensor("out", [R, DM], BF16, kind="ExternalOutput")

    scale = float(DH ** -0.5)

    with tile.TileContext(nc) as tc:
        with tc.tile_pool(name="persist", bufs=1) as pp:
            wq_sb = pp.tile([128, KT, CPC], BF16, tag="wq")
            wk_sb = pp.tile([128, KT, CPC], BF16, tag="wk")
            wv_sb = pp.tile([128, KT, CPC], BF16, tag="wv")
            prot_sb = pp.tile([128, 128], BF16, tag="prot")
            cost_sb = pp.tile([CPC, N], BF16, tag="cost")
            sint_sb = pp.tile([CPC, N], BF16, tag="sint")
            maskb_sb = pp.tile([128, R // 128], F32, tag="maskb")
            qt_sb = pp.tile([CPC, R], BF16, tag="qt")
            kt_sb = pp.tile([CPC, R], BF16, tag="kt")
            vaug_sb = pp.tile([128, (R // 128) * VAUGW], BF16, tag="vaug")
            wo_sb = pp.tile([128, DM], BF16, tag="wo")
            ones_sb = pp.tile([128, 128], BF16, tag="ones")
            nc.vector.memset(ones_sb[:], 1.0)

            junk_sb = pp.tile([128, 512], BF16, tag="junk")
            nc.vector.memset(junk_sb[:], 0.001)

            xt_view = xt_d.ap().rearrange("p (rb k n) -> p rb k n",
                                          rb=RB, k=KT)

            # ALL xt row blocks are SBUF-resident (8 MB); every input DMA
            # is issued up front, striped over the three trigger queues,
            # ordered so the earliest-needed bytes land first.
            xt_all = pp.tile([128, RB, KT, 512], BF16, tag="xtall")
            woB2 = pp.tile([DH, DM], BF16, tag="woB2")
            # Queue order = landing order.  Critical path to the first
            # exp: wq/wk -> rb0 (split over two queues) -> cost/sint
            # first block -> maskb.
            nc.sync.dma_start(wq_sb[:],
                              wq_d.ap().rearrange("p (k n) -> p k n", k=KT))
            nc.scalar.dma_start(wk_sb[:],
                                wk_d.ap().rearrange("p (k n) -> p k n", k=KT))
            nc.gpsimd.dma_start(wv_sb[:],
                                wv_d.ap().rearrange("p (k n) -> p k n", k=KT))
            nc.sync.dma_start(xt_all[:, 0, 0:4], xt_view[:, 0, 0:4])
            nc.gpsimd.dma_start(xt_all[:, 0, 4:8], xt_view[:, 0, 4:8])
            nc.scalar.dma_start(cost_sb[:, 0:512], cost_d[:, 0:512])
            nc.scalar.dma_start(sint_sb[:, 0:512], sint_d[:, 0:512])
            nc.scalar.dma_start(prot_sb[:], prot_d[:, :])
            # pre-load the ACT Exp table during the initial DMA wait
            warm_sb = pp.tile([1, 2], F32, tag="warm")
            nc.vector.memset(warm_sb[:], 0.0)
            nc.scalar.activation(warm_sb[0:1, 1:2], warm_sb[0:1, 0:1],
                                 mybir.ActivationFunctionType.Exp)
            nc.sync.dma_start(maskb_sb[:], maskb_d[:, :])
            ones_view = vaug_sb[:].rearrange("p (t u w) -> p (t u) w",
                                             u=2, w=DH + 1)[:, :, DH]
            nc.gpsimd.dma_start(ones_view, vones_d[:, :])
            nc.sync.dma_start(xt_all[:, 1], xt_view[:, 1])
            nc.gpsimd.dma_start(xt_all[:, 2], xt_view[:, 2])
            nc.scalar.dma_start(xt_all[:, 3], xt_view[:, 3])
            nc.scalar.dma_start(cost_sb[:, 512:], cost_d[:, 512:])
            nc.scalar.dma_start(sint_sb[:, 512:], sint_d[:, 512:])
            nc.sync.dma_start(xt_all[:, 4], xt_view[:, 4])
            nc.gpsimd.dma_start(xt_all[:, 5], xt_view[:, 5])
            nc.sync.dma_start(wo_sb[:], wout_d[:, :])
            nc.gpsimd.dma_start(woB2[:], wout_d[DH:2 * DH, :])
            nc.scalar.dma_start(xt_all[:, 6], xt_view[:, 6])
            nc.sync.dma_start(xt_all[:, 7], xt_view[:, 7])

            with tc.tile_pool(name="p1", bufs=3) as p1, \
                 tc.tile_pool(name="psc", bufs=2, space="PSUM") as psc, \
                 tc.tile_pool(name="p2", bufs=3) as p2, \
                 tc.tile_pool(name="ppt", bufs=6) as ppt, \
                 tc.tile_pool(name="ps_sc", bufs=2, space="PSUM") as ps_sc, \
                 tc.tile_pool(name="ps_o", bufs=1, space="PSUM") as ps_o:

                # Warm-up stream: dependency-free junk matmuls bridge the
                # initial input-DMA wait so the PE HAM clock is at 8/8
                # when the first projection matmul issues.  All into ONE
                # tile: same-engine WAW needs no semaphore, so they run
                # back-to-back.
                JW = int(os.environ.get("BASS_JW", "30"))
                jp = psc.tile([128, 512], F32, tag="c", name="junk")
                for _ in range(JW):
                    nc.tensor.matmul(jp[:], ones_sb[:], junk_sb[:],
                                     start=True, stop=True)

                def rb_fillers(rb):
                    """Projection + rotary + v_aug for one 512-row block,
                    split into small chunks so they can be woven between
                    a pass's key-tile groups."""
                    c0 = rb * 512
                    st = {}

                    def f_start():
                        st['xt'] = xt_all[:, rb]
                        st['q'] = psc.tile([128, 512], F32, tag="c", name="q")

                    def f_q(k0):
                        def f():
                            for kt in range(k0, k0 + 4):
                                nc.tensor.matmul(
                                    st['q'][:], wq_sb[:, kt, :],
                                    st['xt'][:, kt, :],
                                    start=(kt == 0), stop=(kt == KT - 1))
                            if k0 + 4 == KT:
                                st['qraw'] = p1.tile([128, 512], BF16,
                                                     tag="qraw", name="qraw")
                                nc.vector.tensor_copy(st['qraw'][:],
                                                      st['q'][:])
                        return f

                    def f_k(k0):
                        def f():
                            if k0 == 0:
                                st['k'] = psc.tile([128, 512], F32, tag="c",
                                                   name="k")
                            for kt in range(k0, k0 + 4):
                                nc.tensor.matmul(
                                    st['k'][:], wk_sb[:, kt, :],
                                    st['xt'][:, kt, :],
                                    start=(kt == 0), stop=(kt == KT - 1))
                            if k0 + 4 == KT:
                                st['kraw'] = p1.tile([128, 512], BF16,
                                                     tag="kraw", name="kraw")
                                nc.vector.tensor_copy(st['kraw'][:],
                                                      st['k'][:])
                        return f

                    def f_v(k0):
                        def f():
                            if k0 == 0:
                                st['v'] = psc.tile([128, 512], F32, tag="c",
                                                   name="v")
                            for kt in range(k0, k0 + 4):
                                for vt in range(4):
                                    nc.tensor.matmul(
                                        st['v'][:, vt * 128:(vt + 1) * 128],
                                        st['xt'][:, kt, vt * 128:(vt + 1) * 128],
                                        wv_sb[:, kt, :],
                                        start=(kt == 0 and vt == 0),
                                        stop=(kt == KT - 1))
                            if k0 + 4 == KT:
                                kt0 = rb * 4
                                va = vaug_sb[:].rearrange("p (t w) -> p t w",
                                                          w=VAUGW)
                                vp = st['v'][:].rearrange("p (t c) -> p t c",
                                                          c=128)
                                nc.vector.tensor_copy(
                                    va[:, kt0:kt0 + 4, 0:DH], vp[:, :, 0:DH])
                                nc.vector.tensor_copy(
                                    va[:, kt0:kt0 + 4, DH + 1:DH + 1 + DH],
                                    vp[:, :, DH:2 * DH])
                        return f

                    def f_rot(dst, rawkey):
                        def f():
                            raw = st[rawkey]
                            rot_ps = psc.tile([128, 512], F32, tag="c",
                                              name="rot")
                            nc.tensor.matmul(rot_ps[:], prot_sb[:], raw[:],
                                             start=True, stop=True)
                            cc = c0 % N
                            dv = dst[:, c0:c0 + 512]
                            tmp = p1.tile([128, 512], BF16, tag="rottmp")
                            nc.vector.tensor_mul(dv, raw[:],
                                                 cost_sb[:, cc:cc + 512])
                            nc.vector.tensor_mul(tmp[:], rot_ps[:],
                                                 sint_sb[:, cc:cc + 512])
                            nc.vector.tensor_add(dv, dv, tmp[:])
                        return f

                    def f_first():
                        f_start()
                        f_q(0)()
                    return [f_first, f_q(4), f_k(0), f_k(4),
                            f_v(0), f_v(4),
                            f_rot(qt_sb, 'qraw'), f_rot(kt_sb, 'kraw')]

                def emit_rb(rb):
                    for f in rb_fillers(rb):
                        f()

                Y_ENG = {0: nc.sync, 1: nc.gpsimd, 2: nc.sync, 3: nc.gpsimd}

                def tail_fillers(j, o_ps):
                    """Normalization + local output projection for a
                    finished pass.  First chunk releases the o PSUM
                    banks (it holds every read of o_ps), so it MUST be
                    emitted before the next pass's first attnV.  All
                    engine ops keep in/out base partitions aligned; the
                    head-B 64->128 partition stack goes through one
                    small SBUF->SBUF DMA."""
                    rows0 = j * QC
                    st = {}

                    def f_readout():
                        st['rcp'] = []
                        st['onum'] = []
                        for h in range(HPC):
                            rcp = p2.tile([DH + 1, QC], F32, tag=f"rcp{h}",
                                          name=f"rcp{h}")
                            nc.vector.reciprocal_approx_fast(rcp[:], o_ps[h][:])
                            st['rcp'].append(rcp)
                            onum = p2.tile([DH, QC], BF16, tag=f"on{h}",
                                           name=f"on{h}")
                            nc.vector.tensor_copy(onum[:], o_ps[h][0:DH, :])
                            st['onum'].append(onum)

                    def f_div():
                        st['onb'] = p2.tile([128, QC], BF16, tag="onb2",
                                            name="onb2")
                        for h in range(HPC):
                            rcpb = p2.tile([DH + 1, QC], BF16, tag=f"rb{h}",
                                           name=f"rb{h}")
                            nc.vector.tensor_copy(rcpb[DH:DH + 1, :],
                                                  st['rcp'][h][DH:DH + 1, :])
                            div_ps = psc.tile([128, QC], F32, tag="c",
                                              name="div")
                            nc.tensor.matmul(div_ps[:], ones_sb[DH:DH + 1, :],
                                             rcpb[DH:DH + 1, :],
                                             start=True, stop=True,
                                             tile_position=(64, 0))
                            div_sb = p2.tile([DH, QC], BF16, tag=f"dv{h}",
                                             name=f"dv{h}")
                            nc.vector.tensor_copy(div_sb[:], div_ps[0:DH, :])
                            if h == 0:
                                nc.vector.tensor_mul(
                                    st['onb'][0:DH, :], st['onum'][0][:],
                                    div_sb[:])
                            else:
                                onbB = p2.tile([DH, QC], BF16, tag="onbB",
                                               name="onbB")
                                nc.vector.tensor_mul(onbB[:], st['onum'][1][:],
                                                     div_sb[:])
                                nc.gpsimd.dma_start(
                                    st['onb'][DH:2 * DH, :], onbB[:])

                    def f_y(i):
                        def f():
                            ysb = p2.tile([128, DM], BF16, tag="ysb",
                                          name="ysb")
                            for ob in range(2):
                                yp = psc.tile([128, 512], F32, tag="c",
                                              name="y")
                                nc.tensor.matmul(
                                    yp[:],
                                    st['onb'][:, i * 128:(i + 1) * 128],
                                    wo_sb[:, ob * 512:(ob + 1) * 512],
                                    start=True, stop=True)
                                nc.vector.tensor_copy(
                                    ysb[:, ob * 512:(ob + 1) * 512], yp[:])
                            r0 = rows0 + i * 128
                            Y_ENG[i].dma_start(out_d[r0:r0 + 128, :], ysb[:])
                        return f

                    return [f_readout, f_div, f_y(0), f_y(1), f_y(2), f_y(3)]

                def tail_last(j, o_ps):
                    """Chunked tail for the final pass: nothing overlaps
                    it, so shorten the serial chain.  Per-head K=64
                    output matmuls (accumulating pairs against
                    wo_sb[0:64] / woB2) avoid the SBUF->SBUF partition
                    stack DMA entirely."""
                    rows0 = j * QC
                    st = {}

                    def f_readout():
                        st['rcp'] = []
                        st['onum'] = []
                        for h in range(HPC):
                            rcp = p2.tile([DH + 1, QC], F32, tag=f"rcp{h}",
                                          name=f"rcp{h}")
                            nc.vector.reciprocal_approx_fast(rcp[:], o_ps[h][:])
                            st['rcp'].append(rcp)
                            onum = p2.tile([DH, QC], BF16, tag=f"on{h}",
                                           name=f"on{h}")
                            nc.vector.tensor_copy(onum[:], o_ps[h][0:DH, :])
                            st['onum'].append(onum)

                    def f_norm():
                        st['onb'] = []
                        for h in range(HPC):
                            rcpb = p2.tile([DH + 1, QC], BF16, tag=f"rb{h}",
                                           name=f"rb{h}")
                            nc.vector.tensor_copy(rcpb[DH:DH + 1, :],
                                                  st['rcp'][h][DH:DH + 1, :])
                            div_ps = psc.tile([128, QC], F32, tag="c",
                                              name="div")
                            nc.tensor.matmul(div_ps[:], ones_sb[DH:DH + 1, :],
                                             rcpb[DH:DH + 1, :],
                                             start=True, stop=True,
                                             tile_position=(64, 0))
                            div_sb = p2.tile([DH, QC], BF16, tag=f"dv{h}",
                                             name=f"dv{h}")
                            nc.vector.tensor_copy(div_sb[:], div_ps[0:DH, :])
                            onb = p2.tile([DH, QC], BF16, tag=f"onbl{h}",
                                          name=f"onbl{h}")
                            nc.vector.tensor_mul(onb[:], st['onum'][h][:],
                                                 div_sb[:])
                            st['onb'].append(onb)

                    def f_ych(i):
                        def f():
                            ysb = p2.tile([128, DM], BF16, tag="ysb",
                                          name="ysb")
                            r0 = rows0 + i * 128
                            for ob in range(2):
                                yp = psc.tile([128, 512], F32, tag="c",
                                              name="y")
                                nc.tensor.matmul(
                                    yp[:],
                                    st['onb'][0][:, i * 128:(i + 1) * 128],
                                    wo_sb[0:DH, ob * 512:(ob + 1) * 512],
                                    start=True, stop=False)
                                nc.tensor.matmul(
                                    yp[:],
                                    st['onb'][1][:, i * 128:(i + 1) * 128],
                                    woB2[:, ob * 512:(ob + 1) * 512],
                                    start=False, stop=True)
                                nc.vector.tensor_copy(
                                    ysb[:, ob * 512:(ob + 1) * 512], yp[:])
                                # half-row DMA as soon as each cast lands
                                Y_ENG[(2 * i + ob) % 2].dma_start(
                                    out_d[r0:r0 + 128,
                                          ob * 512:(ob + 1) * 512],
                                    ysb[:, ob * 512:(ob + 1) * 512])
                        return f

                    def f_warm():
                        # keep the PE HAM clock at 8/8 through the
                        # DVE-only normalization window so the final
                        # output-projection matmuls run at full rate
                        tj = psc.tile([128, 512], F32, tag="c", name="tj")
                        for _ in range(16):
                            nc.tensor.matmul(tj[:], ones_sb[:], junk_sb[:],
                                             start=True, stop=True)

                    return [f_warm, f_readout, f_norm, f_ych(0), f_ych(1),
                            f_ych(2), f_ych(3)]

                # ---- unified software-pipelined attention stream ----
                # One global loop over key-tile index G (pass p = G//16).
                # Per-G emission order: score(G) -> exp(G) -> fillers(G)
                # -> attnV(G-L).  attnV lags L=4 iterations behind the
                # score/exp front so the next pass's first score matmuls
                # sit ahead of the previous pass's attnV backlog in the
                # PE queue and the exp stream stays dense across pass
                # boundaries.
                L = 4
                GT = B * NPASS * NKEYT  # 128
                sched = {}

                def add(G, fs):
                    sched.setdefault(G, []).extend(fs)

                def pack(G0, chunks, sizes):
                    i = 0
                    g = G0
                    for s in sizes:
                        grp = chunks[i:i + s]
                        i += s
                        if not grp:
                            break

                        def runner(grp=grp):
                            for f in grp:
                                f()
                        add(g, [runner])
                        g += 1
                    assert i >= len(chunks)

                # rb1-4 woven into pass 0 (rb_i complete before G=4*i);
                # rb5-7 into passes 1-3, avoiding each pass's last slot
                # and the tail slots (4..9).
                pack(1, rb_fillers(1), [3, 3, 2])
                pack(4, rb_fillers(2), [2, 2, 2, 2])
                pack(8, rb_fillers(3), [2, 2, 2, 2])
                pack(12, rb_fillers(4), [2, 2, 2, 2])
                for i, rb in enumerate((5, 6, 7)):
                    base = 16 * (i + 1)
                    ch = rb_fillers(rb)
                    pack(base + 1, ch[0:3], [1, 1, 1])
                    pack(base + 10, ch[3:8], [1, 1, 1, 1, 1])

                o_ps_map = {}
                pt_hist = {}

                def emit_score(p, k):
                    b, qc = divmod(p, NPASS)
                    qb = b * N + qc * QC
                    g = b * NKEYT + k
                    krow = b * N + k * 128
                    sc = ps_sc.tile([128, 2 * QC], F32, tag="sc", name="sc")
                    for h in range(HPC):
                        ho = h * DH
                        nc.tensor.matmul(
                            sc[:, h * QC:(h + 1) * QC],
                            kt_sb[ho:ho + DH, krow:krow + 128],
                            qt_sb[ho:ho + DH, qb:qb + QC],
                            start=True, stop=True)
                    pt = ppt.tile([128, 2 * QC], BF16, tag="p", name="pt")
                    nc.scalar.activation(
                        pt[:], sc[:], mybir.ActivationFunctionType.Exp,
                        bias=maskb_sb[:, g:g + 1], scale=scale)
                    pt_hist[p * NKEYT + k] = pt

                def emit_attnv(p, k):
                    b, qc = divmod(p, NPASS)
                    gp = b * NKEYT + k
                    pt = pt_hist.pop(p * NKEYT + k)
                    for h in range(HPC):
                        va_l = vaug_sb[:, gp * VAUGW + h * (DH + 1):
                                       gp * VAUGW + (h + 1) * (DH + 1)]
                        nc.tensor.matmul(
                            o_ps_map[p][h][:], va_l,
                            pt[:, h * QC:(h + 1) * QC],
                            start=(k == 0), stop=(k == NKEYT - 1))

                emit_rb(0)
                for G in range(GT + L + 8):
                    p, k = divmod(G, NKEYT)
                    if G < GT:
                        if k == 0:
                            o_ps_map[p] = [
                                ps_o.tile([DH + 1, QC], F32, tag=f"o{h}",
                                          name=f"o{h}") for h in range(HPC)]
                            # tail of pass p-1 into slots L..L+5 of pass p
                            if p > 0:
                                for i, f in enumerate(
                                        tail_fillers(p - 1, o_ps_map[p - 1])):
                                    add(G + L + i, [f])
                        emit_score(p, k)
                    if G == GT:
                        for i, f in enumerate(
                                tail_last(NPT - 1, o_ps_map[NPT - 1])):
                            add(G + L + i, [f])
                    for f in sched.pop(G, ()):
                        f()
                    if 0 <= G - L < GT:
                        p2_, k2 = divmod(G - L, NKEYT)
                        emit_attnv(p2_, k2)

    nc.compile()
    return nc


_NC_CACHE = None


def kernel(x, mask, pos_emb, Wq, Wkv, Wout, bout):
    global LAST_EXEC_TIME_NS, LAST_TRACE_DIR, _NC_CACHE

    x = np.asarray(x, dtype=np.float32)
    mask = np.asarray(mask)
    pos_emb = np.asarray(pos_emb, dtype=np.float32)
    Wq = np.asarray(Wq, dtype=np.float32)
    Wkv = np.asarray(Wkv, dtype=np.float32)
    Wout = np.asarray(Wout, dtype=np.float32)
    bout = np.asarray(bout, dtype=np.float32)

    bf = ml_dtypes.bfloat16
    # xt2[p, ((rb*KT)+kt)*512+n] = x[rb*512+n, kt*128+p]: each partition
    # line is 8 KB contiguous per row-block -> fast DMA.
    xt2 = np.ascontiguousarray(
        x.reshape(RB, 512, KT, 128).transpose(3, 0, 2, 1)
        .reshape(128, RB * KT * 512)).astype(bf)

    def wprep(w):
        # w2[p, kt*CPC+m] = w[kt*128+p, m] (2 KB contiguous lines)
        return np.ascontiguousarray(
            w.reshape(KT, 128, CPC).transpose(1, 0, 2)
            .reshape(128, KT * CPC)).astype(bf)

    wk_full = Wkv[:, :H * DH]
    wv_full = Wkv[:, H * DH:]
    cost = np.ascontiguousarray(np.tile(np.cos(pos_emb).T, (HPC, 1))).astype(bf)
    sint = np.ascontiguousarray(np.tile(np.sin(pos_emb).T, (HPC, 1))).astype(bf)
    maskb = np.ascontiguousarray(
        np.where(mask.reshape(R), 0.0, -1e5).astype(np.float32)
        .reshape(R // 128, 128).T)
    # rot2 as a matmul: rot2(q) = P @ q (q in [chan, row] layout);
    # lhsT for the tensor engine is P.T
    prot = np.zeros((128, 128), dtype=bf)
    for i in range(64):
        prot[2 * i + 1, 2 * i] = -1.0
        prot[2 * i, 2 * i + 1] = 1.0

    in_maps = []
    for c in range(NCORES):
        cols = slice(c * CPC, (c + 1) * CPC)
        in_maps.append({
            "xt": xt2,
            "wq": wprep(Wq[:, cols]),
            "wk": wprep(wk_full[:, cols]),
            "wv": wprep(wv_full[:, cols]),
            "prot": prot,
            "wout": np.ascontiguousarray(Wout[cols, :]).astype(bf),
            "cost": cost,
            "sint": sint,
            "maskb": maskb,
            "vones": np.ones((128, (R // 128) * 2), dtype=bf),
        })

    dbg = bool(int(os.environ.get("BASS_KERNEL_DEBUG", "0")))
    if _NC_CACHE is None:
        _NC_CACHE = build(dbg=dbg)
    nc = _NC_CACHE

    trace = bool(int(os.environ.get("BASS_KERNEL_TRACE", "0")))
    kwargs = {}
    if trace:
        _install_trace_shim()
        tdir = os.environ.get("BASS_TRACE_DIR", "/tmp/bass_trace_out")
        import shutil
        shutil.rmtree(tdir, ignore_errors=True)
        os.makedirs(tdir, exist_ok=True)
        kwargs["tmpdir"] = tdir
    res = bass_utils.run_bass_kernel_spmd(
        nc, in_maps, core_ids=list(range(NCORES)), trace=trace, **kwargs)
    LAST_EXEC_TIME_NS = res.exec_time_ns
    if res.instructions_and_trace is not None:
        LAST_TRACE_DIR = res.instructions_and_trace[1]
        globals()["LAST_INSTS"] = res.instructions_and_trace[0]

    globals()["LAST_RESULTS"] = res.results
    y = np.zeros((R, DM), dtype=np.float32)
    for c in range(NCORES):
        y += res.results[c]["out"].astype(np.float32)
    y += bout[None, :]
    return y.reshape(B, N, DM)
